# revision 9
# baseline (speedup 1.0000x reference)
"""Mean aggregation over sampled neighbors (GNN message passing) on 8 TRN2 cores.

reference:  out[n, :] = mean_j feature[neighbor_idx[n, j], :]
  feature      [200000, 64]  f32
  neighbor_idx [100000, 12]  int
  out          [100000, 64]  f32

Strategy: shard n_nodes across the 8 cores (12500 nodes each); replicate the
feature table into every core's HBM. Each core processes its nodes in tiles
of 128 (one node per SBUF partition). For each tile it issues 12 indirect
DMAs (SWDGE gather, one offset per partition) fetching neighbor j's feature
row for all 128 nodes, accumulates the 12 gathered tiles on the vector
engine, scales by 1/12, and streams the results out.

Note: this runtime exposes no batched-index gather (the extended GPSIMD
dma_gather ucode is unavailable, and indirect DMA consumes exactly one
offset per dest partition), so the gather rate is bound by the SWDGE
per-instruction overhead (~1.5us per 128 rows, measured).
"""

import sys

sys.path.insert(0, "/opt/trn_rl_repo")

import numpy as np

import concourse.bacc as bacc
import concourse.bass as bass
import concourse.tile as tile
from concourse import mybir
from concourse.bass_utils import run_bass_kernel_spmd

P = 128             # SBUF partitions = nodes per tile
N_TOTAL = 200000    # feature table rows
D = 64              # feature dim
N_NODES = 100000
S = 12              # sampled neighbors per node
N_CORES = 8
NODES_PER_CORE = N_NODES // N_CORES          # 12500
N_TILES = -(-NODES_PER_CORE // P)            # 98 node tiles of 128
NODES_PAD = N_TILES * P                      # 12544

_cached = {}


def _build_program():
    nc = bacc.Bacc("TRN2", target_bir_lowering=False)
    feat = nc.dram_tensor("feature", [N_TOTAL, D], mybir.dt.float32,
                          kind="ExternalInput").ap()
    idxt = nc.dram_tensor("idx_t", [P, N_TILES * S], mybir.dt.int32,
                          kind="ExternalInput").ap()
    out = nc.dram_tensor("out", [N_TILES, P, D], mybir.dt.float32,
                         kind="ExternalOutput").ap()

    with tile.TileContext(nc) as tc:
        with tc.tile_pool(name="sbuf", bufs=3) as pool:
            # One DMA for every offset: all later waits on it are satisfied
            # after the first gather, so Tile stops emitting Pool-side waits.
            offs_all = pool.tile([P, N_TILES * S], mybir.dt.int32, tag="offs")
            nc.sync.dma_start(out=offs_all[:], in_=idxt[:])
            for t in range(N_TILES):
                gs = []
                for j in range(S):
                    g = pool.tile([P, D], mybir.dt.float32, tag=f"g{j}")
                    nc.gpsimd.indirect_dma_start(
                        out=g[:],
                        out_offset=None,
                        in_=feat[:],
                        in_offset=bass.IndirectOffsetOnAxis(
                            ap=offs_all[:, t * S + j:t * S + j + 1], axis=0),
                    )
                    gs.append(g)
                acc = pool.tile([P, D], mybir.dt.float32, tag="acc")
                nc.vector.tensor_add(acc[:], gs[0][:], gs[1][:])
                for j in range(2, S):
                    nc.vector.tensor_add(acc[:], acc[:], gs[j][:])
                st = pool.tile([P, D], mybir.dt.float32, tag="st")
                nc.vector.tensor_scalar_mul(st[:], acc[:], 1.0 / S)
                nc.sync.dma_start(out=out[t], in_=st[:])
    nc.compile()
    return nc


def _prep_idx(nbr_shard):
    """[NODES_PER_CORE, S] int -> [P, N_TILES*S] int32 (padded with row 0).

    Layout: [p, t*S + j] = idx of neighbor j of node t*128+p, so the whole
    offsets table loads into SBUF with one contiguous DMA."""
    padded = np.zeros((NODES_PAD, S), dtype=np.int32)
    padded[:NODES_PER_CORE] = nbr_shard
    return np.ascontiguousarray(
        padded.reshape(N_TILES, P, S).transpose(1, 0, 2).reshape(P, N_TILES * S)
    )


def kernel(feature, neighbor_idx, _trace=False, **_run_kwargs):
    feature = np.ascontiguousarray(np.asarray(feature), dtype=np.float32)
    nbr32 = np.asarray(neighbor_idx).astype(np.int32)

    if "nc" not in _cached:
        _cached["nc"] = _build_program()
    nc = _cached["nc"]

    in_maps = [
        {
            "feature": feature,
            "idx_t": _prep_idx(nbr32[c * NODES_PER_CORE:(c + 1) * NODES_PER_CORE]),
        }
        for c in range(N_CORES)
    ]
    res = run_bass_kernel_spmd(
        nc, in_maps, core_ids=list(range(N_CORES)), trace=_trace, **_run_kwargs
    )

    outs = []
    for c in range(N_CORES):
        o = res.results[c]["out"].reshape(NODES_PAD, D)
        outs.append(o[:NODES_PER_CORE])
    full = np.concatenate(outs, axis=0)
    if _trace:
        return full, res
    return full


# revision 10
# speedup vs baseline: 1.0107x; 1.0107x over previous
"""Mean aggregation over sampled neighbors (GNN message passing) on 8 TRN2 cores.

reference:  out[n, :] = mean_j feature[neighbor_idx[n, j], :]
  feature      [200000, 64]  f32
  neighbor_idx [100000, 12]  int
  out          [100000, 64]  f32

Strategy: shard n_nodes across the 8 cores (12500 nodes each); replicate the
feature table into every core's HBM. Each core processes its nodes in tiles
of 128 (one node per SBUF partition). For each tile it issues 12 indirect
DMAs (SWDGE gather, one offset per partition) fetching neighbor j's feature
row for all 128 nodes, accumulates the 12 gathered tiles on the vector
engine, scales by 1/12, and streams the results out.

Note: this runtime exposes no batched-index gather (the extended GPSIMD
dma_gather ucode is unavailable, and indirect DMA consumes exactly one
offset per dest partition), so the gather rate is bound by the SWDGE
per-instruction overhead (~1.5us per 128 rows, measured).
"""

import sys

sys.path.insert(0, "/opt/trn_rl_repo")

import numpy as np

import concourse.bacc as bacc
import concourse.bass as bass
import concourse.tile as tile
from concourse import mybir
from concourse.bass_utils import run_bass_kernel_spmd

P = 128             # SBUF partitions = nodes per tile
N_TOTAL = 200000    # feature table rows
D = 64              # feature dim
N_NODES = 100000
S = 12              # sampled neighbors per node
N_CORES = 8
NODES_PER_CORE = N_NODES // N_CORES          # 12500
N_TILES = -(-NODES_PER_CORE // P)            # 98 node tiles of 128
NODES_PAD = N_TILES * P                      # 12544

_cached = {}


def _build_program():
    nc = bacc.Bacc("TRN2", target_bir_lowering=False)
    feat = nc.dram_tensor("feature", [N_TOTAL, D], mybir.dt.float32,
                          kind="ExternalInput").ap()
    idxt = nc.dram_tensor("idx_t", [P, N_TILES * S], mybir.dt.int32,
                          kind="ExternalInput").ap()
    out = nc.dram_tensor("out", [N_TILES, P, D], mybir.dt.float32,
                         kind="ExternalOutput").ap()

    with tile.TileContext(nc) as tc:
        with tc.tile_pool(name="sbuf", bufs=3) as pool:
            # One DMA for every offset: all later waits on it are satisfied
            # after the first gather, so Tile stops emitting Pool-side waits.
            offs_all = pool.tile([P, N_TILES * S], mybir.dt.int32, tag="offs")
            nc.sync.dma_start(out=offs_all[:], in_=idxt[:])
            for t in range(N_TILES):
                # 12 gathers land in disjoint 64-col slices of ONE tile; a
                # single strided tensor_reduce consumes all of them, so the
                # 12 WAW waits per tile collapse onto one DVE tick.
                g = pool.tile([P, S * D], mybir.dt.float32, tag="g")
                for j in range(S):
                    nc.gpsimd.indirect_dma_start(
                        out=g[:, j * D:(j + 1) * D],
                        out_offset=None,
                        in_=feat[:],
                        in_offset=bass.IndirectOffsetOnAxis(
                            ap=offs_all[:, t * S + j:t * S + j + 1], axis=0),
                    )
                st = pool.tile([P, D], mybir.dt.float32, tag="st")
                # view [P, D, S]: reduce the neighbor axis (stride D) innermost
                nc.vector.tensor_reduce(
                    out=st[:].rearrange("p d -> p d", d=D),
                    in_=g[:].rearrange("p (s d) -> p d s", s=S, d=D),
                    axis=mybir.AxisListType.X,
                    op=mybir.AluOpType.add,
                )
                nc.vector.tensor_scalar_mul(st[:], st[:], 1.0 / S)
                nc.sync.dma_start(out=out[t], in_=st[:])
    nc.compile()
    return nc


def _prep_idx(nbr_shard):
    """[NODES_PER_CORE, S] int -> [P, N_TILES*S] int32 (padded with row 0).

    Layout: [p, t*S + j] = idx of neighbor j of node t*128+p, so the whole
    offsets table loads into SBUF with one contiguous DMA."""
    padded = np.zeros((NODES_PAD, S), dtype=np.int32)
    padded[:NODES_PER_CORE] = nbr_shard
    return np.ascontiguousarray(
        padded.reshape(N_TILES, P, S).transpose(1, 0, 2).reshape(P, N_TILES * S)
    )


def kernel(feature, neighbor_idx, _trace=False, **_run_kwargs):
    feature = np.ascontiguousarray(np.asarray(feature), dtype=np.float32)
    nbr32 = np.asarray(neighbor_idx).astype(np.int32)

    if "nc" not in _cached:
        _cached["nc"] = _build_program()
    nc = _cached["nc"]

    in_maps = [
        {
            "feature": feature,
            "idx_t": _prep_idx(nbr32[c * NODES_PER_CORE:(c + 1) * NODES_PER_CORE]),
        }
        for c in range(N_CORES)
    ]
    res = run_bass_kernel_spmd(
        nc, in_maps, core_ids=list(range(N_CORES)), trace=_trace, **_run_kwargs
    )

    outs = []
    for c in range(N_CORES):
        o = res.results[c]["out"].reshape(NODES_PAD, D)
        outs.append(o[:NODES_PER_CORE])
    full = np.concatenate(outs, axis=0)
    if _trace:
        return full, res
    return full
